# revision 22
# baseline (speedup 1.0000x reference)
"""KAN basis-linear kernel for 8 TRN2 NeuronCores (mixed bf16/fp8 v3).

Computes, for x:[B,I], spline_weight:[O,I,K=9], base_weight:[O,I], bias:[O]:

    basis = relu(1 - |(clip(x,-2,2)[...,None] - grid) / delta|)   # hat basis
    out   = einsum('bik,oik->bo', basis, spline_weight)
          + silu(x) @ base_weight.T + bias

Strategy: data-parallel over the batch across 8 cores (weights replicated).

Algebra: contract the LOCAL hat basis directly (not the telescoped ramp
basis): hat_k = psi_k - psi_{k-1} where psi_j = clip(2*(g_{j+1}-x),0,1)
(saturation subsumes the x-clip; hat_0 = psi_0, hat_8 = relu(1 - r_7)).
Partition of unity (sum_k hat_k = 1) folds channel k=4 into the bias
(weights W_k = sw_k - sw_4), leaving 8 hat channels + silu = 9
contraction channels.

Mixed precision: the two lowest-energy channels (hat_0, hat_8; E[hat^2]
~ 0.034 each vs 0.68 total) are cast to fp8-e4m3 on both sides and run
as ONE DoubleRow matmul per i-chunk pair (2 contraction planes per
instruction at the same 1-moving-row/cycle stream rate), so PE work is
7 + 2/2 = 8 bf16-equivalent channel-units instead of the baseline's 9
(-11%). Offline numpy sim of the exact quantization (sim_fp8b.py)
measures gate error 1.40e-2 vs the 2e-2 limit; device measures 1.43e-2.

Schedule: software-pipelined one (super, pair) step ahead - production
(ACT ramps + DVE min/sub) and weight DMAs for step i+1 are emitted
before the matmuls of step i, so phi for the next super exists before
the super boundary and PE never waits on production. The last pair of
each super orders matmuls ot-outer so the 8 PSUM banks finish staggered
(3.5us apart); evacuation (DVE, +bias) follows each stop and the output
DMAs ride the otherwise-idle GpSimd queue to keep the scalar/sync
queues free for x/weight loads.
"""
import numpy as np
import ml_dtypes
from contextlib import ExitStack

import concourse.bass as bass
import concourse.tile as tile
import concourse.mybir as mybir
from concourse import bacc
from concourse.bass_utils import run_bass_kernel_spmd

N_CORES = 8
B, I, O, K = 16384, 1024, 1024, 9
B_CORE = B // N_CORES            # 2048 batch rows per core
B_SUPER = 512                    # batch stripe held in PSUM (1 bank per o-tile)
N_SUPERS = B_CORE // B_SUPER     # 4
P = 128
N_ICHK = I // P                  # 8 contraction chunks over i
N_PAIR = N_ICHK // 2             # 4 fp8 DoubleRow pairs
N_OT = O // P                    # 8 output tiles (one PSUM bank each)
NB = 7                           # bf16 planes: hat{1,2,3,5,6,7} + silu
BF_HATS = [1, 2, 3, 5, 6, 7]

F32 = mybir.dt.float32
BF16 = mybir.dt.bfloat16
FP8 = mybir.dt.float8e4
AF = mybir.ActivationFunctionType
DR = mybir.MatmulPerfMode.DoubleRow

_CACHE = {}


def _build():
    nc = bacc.Bacc("TRN2", target_bir_lowering=False, debug=False,
                   num_devices=N_CORES)
    # x tiled on host: [bs, ichk, p, b]
    xt = nc.dram_tensor("xt", [N_SUPERS, N_ICHK, P, B_SUPER], F32,
                        kind="ExternalInput").ap()
    # bf16 weights: [ichk, p, c, o], c in (hat1,hat2,hat3,hat5,hat6,hat7,silu)
    wb = nc.dram_tensor("wb", [N_ICHK, P, NB, O], BF16,
                        kind="ExternalInput").ap()
    # fp8 weights: [pair, p, iik, ch, o], ch in (hat0, hat8)
    w8 = nc.dram_tensor("w8", [N_PAIR, P, 2, 2, O], FP8,
                        kind="ExternalInput").ap()
    bias = nc.dram_tensor("bias", [O], F32, kind="ExternalInput").ap()
    # output tiled: [ot, bs, p, b] (contiguous 256KB stores)
    outT = nc.dram_tensor("outT", [N_OT, N_SUPERS, P, B_SUPER], F32,
                          kind="ExternalOutput").ap()

    with tile.TileContext(nc) as tc, ExitStack() as ctx:
        const_pool = ctx.enter_context(tc.tile_pool(name="const", bufs=1))
        x_pool = ctx.enter_context(tc.tile_pool(name="xin", bufs=4))
        r_pool = ctx.enter_context(tc.tile_pool(name="ramp", bufs=3))
        psi_pool = ctx.enter_context(tc.tile_pool(name="psi", bufs=2))
        phib_pool = ctx.enter_context(tc.tile_pool(name="phib", bufs=5))
        phi8_pool = ctx.enter_context(tc.tile_pool(name="phi8", bufs=3))
        wb_pool = ctx.enter_context(tc.tile_pool(name="wbp", bufs=6))
        w8_pool = ctx.enter_context(tc.tile_pool(name="w8p", bufs=3))
        out_pool = ctx.enter_context(tc.tile_pool(name="outs", bufs=4))
        psum_pool = ctx.enter_context(
            tc.tile_pool(name="psum", bufs=N_OT, space="PSUM"))

        # Warm-up spin first: starts the PE busy-streak (p-state ramp)
        # while the first input DMA + phi production are still in flight.
        # DGE warm-up: a tiny transfer on each input queue absorbs the
        # cold DMA-engine ramp before the first real (critical-path) loads.
        dge_warm = const_pool.tile([2, B_SUPER], F32)
        nc.scalar.dma_start(dge_warm[:], xt[0, 0, 0:2])
        nc.sync.dma_start(dge_warm[:], xt[0, 1, 0:2])

        warm = const_pool.tile([P, 256], BF16)
        nc.gpsimd.memset(warm[:], 0.0)
        warm_ps = psum_pool.tile([P, B_SUPER], F32, tag="psum")
        for _ in range(46):
            nc.tensor.matmul(warm_ps[:, :256], lhsT=warm[:, :P],
                             rhs=warm[:], start=True, stop=True)

        # ACT bias constants: 2*g_{j+1} = j - 3 for j=0..7
        consts = const_pool.tile([P, 8], F32)
        for j in range(8):
            nc.any.memset(consts[:, j:j + 1], float(j - 3))

        # bias[o] -> [128, 8] with o = ot*128 + p; loaded lazily (first
        # needed at the first evacuation, ~110us in) so its scatter-gather
        # DMA never sits in front of the startup-critical loads.
        bias_sb = const_pool.tile([P, N_OT], F32)

        state = {"psums": None, "bias_loaded": False}

        def produce(bs, pair):
            """Weight DMAs + phi production for (bs, pair). Returns tiles."""
            # x loads first: they gate phi production (critical at startup)
            x_sbs = []
            for iik in range(2):
                ichk = 2 * pair + iik
                x_sb = x_pool.tile([P, B_SUPER], F32, tag="xin",
                                   name=f"x_{bs}_{ichk}")
                nc.scalar.dma_start(x_sb[:], xt[bs, ichk])
                x_sbs.append(x_sb)
            wbs = []
            for iik in range(2):
                ichk = 2 * pair + iik
                wb_sb = wb_pool.tile([P, NB, O], BF16, tag="wb",
                                     name=f"wb_{bs}_{ichk}")
                if bs == 0 and ichk < 2:
                    # split: first matmuls start after the ~260KB silu slice
                    nc.sync.dma_start(wb_sb[:, 6:7, :], wb[ichk, :, 6:7, :])
                    nc.sync.dma_start(wb_sb[:, 0:6, :], wb[ichk, :, 0:6, :])
                else:
                    nc.sync.dma_start(wb_sb[:], wb[ichk])
                wbs.append(wb_sb)
            w8_sb = w8_pool.tile([P, 2, 2, O], FP8, tag="w8",
                                 name=f"w8_{bs}_{pair}")
            nc.sync.dma_start(w8_sb[:], w8[pair])

            phi8_sb = phi8_pool.tile([P, 2, 2, B_SUPER], FP8, tag="p8",
                                     name=f"p8_{bs}_{pair}")
            phibs = []
            for iik in range(2):
                ichk = 2 * pair + iik
                x_sb = x_sbs[iik]
                psi = psi_pool.tile([P, 8, B_SUPER], BF16, tag="psi",
                                    name=f"psi_{bs}_{ichk}")
                phib = phib_pool.tile([P, NB, B_SUPER], BF16, tag="pb",
                                      name=f"pb_{bs}_{ichk}")
                # silu first: one ACT op after x lands -> the first super's
                # matmuls can start on the silu plane ~1.5us earlier.
                nc.scalar.activation(phib[:, 6], x_sb[:], AF.Silu)
                for j in range(8):
                    # r_j = relu(2*(g_{j+1} - x)) ; 2*g_{j+1} = j - 3
                    r = r_pool.tile([P, B_SUPER], BF16, tag="r",
                                    name=f"r_{bs}_{ichk}_{j}")
                    nc.scalar.activation(r[:], x_sb[:], AF.Relu,
                                         bias=consts[:, j:j + 1], scale=-2.0)
                    nc.vector.tensor_scalar_min(psi[:, j], r[:], 1.0)
                    if j == 7:
                        # hat_8 = relu(1 - r_7) -> fp8 plane
                        nc.scalar.activation(phi8_sb[:, iik, 1, :], r[:],
                                             AF.Relu, bias=1.0, scale=-1.0)
                # hat_0 = psi_0 -> fp8 plane (cast)
                nc.scalar.activation(phi8_sb[:, iik, 0, :], psi[:, 0],
                                     AF.Copy)
                # interior bf16 hats: hat_k = psi_k - psi_{k-1}
                for c, k in enumerate(BF_HATS):
                    nc.vector.tensor_sub(phib[:, c], psi[:, k], psi[:, k - 1])
                phibs.append(phib)
            return wbs, w8_sb, phibs, phi8_sb

        def matmuls(bs, pair, tiles):
            wbs, w8_sb, phibs, phi8_sb = tiles
            if pair == 0:
                state["psums"] = [
                    psum_pool.tile([P, B_SUPER], F32, tag="psum",
                                   name=f"ps_{bs}_{ot}")
                    for ot in range(N_OT)]
            psums = state["psums"]
            if pair < N_PAIR - 1:
                # plane-outer: earliest start once each phi plane is ready.
                # The very first block leads with silu (ready first).
                for iik in range(2):
                    order = ([6, 0, 1, 2, 3, 4, 5]
                             if bs == 0 and pair == 0 else range(NB))
                    for ci, c in enumerate(order):
                        for ot in range(N_OT):
                            nc.tensor.matmul(
                                psums[ot][:],
                                lhsT=wbs[iik][:, c, bass.ts(ot, P)],
                                rhs=phibs[iik][:, c, :],
                                start=(pair == 0 and iik == 0 and ci == 0),
                                stop=False,
                            )
                for ch in range(2):
                    for ot in range(N_OT):
                        nc.tensor.matmul(
                            psums[ot][:],
                            lhsT=w8_sb[:, :, ch, bass.ts(ot, P)],
                            rhs=phi8_sb[:, :, ch, :],
                            start=False, stop=False, perf_mode=DR,
                        )
            else:
                # last pair: ot-outer so PSUM banks finish staggered;
                # evacuation + output DMA chase each stop.
                if not state["bias_loaded"]:
                    nc.sync.dma_start(bias_sb[:],
                                      bias.rearrange("(ot p) -> p ot", p=P))
                    state["bias_loaded"] = True
                for ot in range(N_OT):
                    for iik in range(2):
                        for c in range(NB):
                            nc.tensor.matmul(
                                psums[ot][:],
                                lhsT=wbs[iik][:, c, bass.ts(ot, P)],
                                rhs=phibs[iik][:, c, :],
                                start=False, stop=False,
                            )
                    for ch in range(2):
                        nc.tensor.matmul(
                            psums[ot][:],
                            lhsT=w8_sb[:, :, ch, bass.ts(ot, P)],
                            rhs=phi8_sb[:, :, ch, :],
                            start=False, stop=(ch == 1), perf_mode=DR,
                        )
                    o_sb = out_pool.tile([P, B_SUPER], F32, tag="outs",
                                         name=f"o_{bs}_{ot}")
                    nc.vector.tensor_scalar_add(o_sb[:], psums[ot][:],
                                                bias_sb[:, ot:ot + 1])
                    nc.gpsimd.dma_start(outT[ot, bs], o_sb[:])

        # ---- software pipeline: produce step i+1 before matmuls of i ----
        sched = [(bs, pair) for bs in range(N_SUPERS)
                 for pair in range(N_PAIR)]
        prev = None
        for step in sched:
            tiles = produce(*step)
            if prev is not None:
                matmuls(prev[0][0], prev[0][1], prev[1])
            prev = (step, tiles)
        matmuls(prev[0][0], prev[0][1], prev[1])

    nc.compile()
    return nc


def _get_nc():
    if "nc" not in _CACHE:
        _CACHE["nc"] = _build()
    return _CACHE["nc"]


def _stage_inputs(x, spline_weight, base_weight, bias):
    """Host-side input staging shared by kernel() and test harnesses."""
    # x[b, i] -> [core, bs, ichk, p, b_super]
    xt = np.ascontiguousarray(
        x.reshape(N_CORES, N_SUPERS, B_SUPER, N_ICHK, P)
        .transpose(0, 1, 3, 4, 2))
    sw4 = spline_weight[..., 4]                              # [O, I]
    # bf16 planes: hat{1,2,3,5,6,7} with sw_k - sw_4, then silu/base weight
    planes = [spline_weight[..., k] - sw4 for k in BF_HATS] + [base_weight]
    wb_full = np.stack(planes, axis=2)                       # [O, I, 7]
    wb_dev = np.ascontiguousarray(
        wb_full.transpose(1, 2, 0)                           # [I, 7, O]
        .reshape(N_ICHK, P, NB, O).astype(ml_dtypes.bfloat16))
    # fp8 planes: hat0/hat8 with sw_k - sw_4 -> [pair, p, iik, ch, o]
    w8_full = np.stack([(spline_weight[..., 0] - sw4).T,
                        (spline_weight[..., 8] - sw4).T], axis=1)  # [I, 2, O]
    w8_dev = np.ascontiguousarray(
        w8_full.reshape(N_PAIR, 2, P, 2, O)
        .transpose(0, 2, 1, 3, 4)                            # [pair, p, iik, ch, o]
        .astype(ml_dtypes.float8_e4m3))
    # bias fold: bias + sum_i sw[o, i, 4]
    bias_dev = (bias + sw4.sum(axis=1)).astype(np.float32)
    return xt, wb_dev, w8_dev, bias_dev


def kernel(x, spline_weight, base_weight, bias):
    x = np.asarray(x, dtype=np.float32)
    spline_weight = np.asarray(spline_weight, dtype=np.float32)
    base_weight = np.asarray(base_weight, dtype=np.float32)
    bias = np.asarray(bias, dtype=np.float32)

    nc = _get_nc()
    xt, wb_dev, w8_dev, bias_dev = _stage_inputs(
        x, spline_weight, base_weight, bias)

    in_maps = [{"xt": np.ascontiguousarray(xt[c]), "wb": wb_dev,
                "w8": w8_dev, "bias": bias_dev} for c in range(N_CORES)]
    res = run_bass_kernel_spmd(nc, in_maps, core_ids=list(range(N_CORES)))

    # outT[ot, bs, p, b] per core -> out[b, o]
    outs = []
    for c in range(N_CORES):
        oc = np.asarray(res.results[c]["outT"])
        outs.append(oc.transpose(1, 3, 0, 2).reshape(B_CORE, O))
    return np.ascontiguousarray(np.concatenate(outs, axis=0),
                                dtype=np.float32)


# revision 23
# speedup vs baseline: 1.0065x; 1.0065x over previous
"""KAN basis-linear kernel for 8 TRN2 NeuronCores (mixed bf16/fp8 v3).

Computes, for x:[B,I], spline_weight:[O,I,K=9], base_weight:[O,I], bias:[O]:

    basis = relu(1 - |(clip(x,-2,2)[...,None] - grid) / delta|)   # hat basis
    out   = einsum('bik,oik->bo', basis, spline_weight)
          + silu(x) @ base_weight.T + bias

Strategy: data-parallel over the batch across 8 cores (weights replicated).

Algebra: contract the LOCAL hat basis directly (not the telescoped ramp
basis): hat_k = psi_k - psi_{k-1} where psi_j = clip(2*(g_{j+1}-x),0,1)
(saturation subsumes the x-clip; hat_0 = psi_0, hat_8 = relu(1 - r_7)).
Partition of unity (sum_k hat_k = 1) folds channel k=4 into the bias
(weights W_k = sw_k - sw_4), leaving 8 hat channels + silu = 9
contraction channels.

Mixed precision: the two lowest-energy channels (hat_0, hat_8; E[hat^2]
~ 0.034 each vs 0.68 total) are cast to fp8-e4m3 on both sides and run
as ONE DoubleRow matmul per i-chunk pair (2 contraction planes per
instruction at the same 1-moving-row/cycle stream rate), so PE work is
7 + 2/2 = 8 bf16-equivalent channel-units instead of the baseline's 9
(-11%). Offline numpy sim of the exact quantization (sim_fp8b.py)
measures gate error 1.40e-2 vs the 2e-2 limit; device measures 1.43e-2.

Schedule: software-pipelined one (super, pair) step ahead - production
(ACT ramps + DVE min/sub) and weight DMAs for step i+1 are emitted
before the matmuls of step i, so phi for the next super exists before
the super boundary and PE never waits on production. The last pair of
each super orders matmuls ot-outer so the 8 PSUM banks finish staggered
(3.5us apart); evacuation (DVE, +bias) follows each stop and the output
DMAs ride the otherwise-idle GpSimd queue to keep the scalar/sync
queues free for x/weight loads.
"""
import numpy as np
import ml_dtypes
from contextlib import ExitStack

import concourse.bass as bass
import concourse.tile as tile
import concourse.mybir as mybir
from concourse import bacc
from concourse.bass_utils import run_bass_kernel_spmd

N_CORES = 8
B, I, O, K = 16384, 1024, 1024, 9
B_CORE = B // N_CORES            # 2048 batch rows per core
B_SUPER = 512                    # batch stripe held in PSUM (1 bank per o-tile)
N_SUPERS = B_CORE // B_SUPER     # 4
P = 128
N_ICHK = I // P                  # 8 contraction chunks over i
N_PAIR = N_ICHK // 2             # 4 fp8 DoubleRow pairs
N_OT = O // P                    # 8 output tiles (one PSUM bank each)
NB = 7                           # bf16 planes: hat{1,2,3,5,6,7} + silu
BF_HATS = [1, 2, 3, 5, 6, 7]

F32 = mybir.dt.float32
BF16 = mybir.dt.bfloat16
FP8 = mybir.dt.float8e4
AF = mybir.ActivationFunctionType
DR = mybir.MatmulPerfMode.DoubleRow

_CACHE = {}


def _build():
    nc = bacc.Bacc("TRN2", target_bir_lowering=False, debug=False,
                   num_devices=N_CORES)
    # x tiled on host: [bs, ichk, p, b]
    xt = nc.dram_tensor("xt", [N_SUPERS, N_ICHK, P, B_SUPER], F32,
                        kind="ExternalInput").ap()
    # bf16 weights: [ichk, p, c, o], c in (hat1,hat2,hat3,hat5,hat6,hat7,silu)
    wb = nc.dram_tensor("wb", [N_ICHK, P, NB, O], BF16,
                        kind="ExternalInput").ap()
    # fp8 weights: [pair, p, iik, ch, o], ch in (hat0, hat8)
    w8 = nc.dram_tensor("w8", [N_PAIR, P, 2, 2, O], FP8,
                        kind="ExternalInput").ap()
    bias = nc.dram_tensor("bias", [O], F32, kind="ExternalInput").ap()
    # output tiled: [ot, bs, p, b] (contiguous 256KB stores)
    outT = nc.dram_tensor("outT", [N_OT, N_SUPERS, P, B_SUPER], F32,
                          kind="ExternalOutput").ap()

    with tile.TileContext(nc) as tc, ExitStack() as ctx:
        const_pool = ctx.enter_context(tc.tile_pool(name="const", bufs=1))
        x_pool = ctx.enter_context(tc.tile_pool(name="xin", bufs=4))
        r_pool = ctx.enter_context(tc.tile_pool(name="ramp", bufs=3))
        psi_pool = ctx.enter_context(tc.tile_pool(name="psi", bufs=2))
        phib_pool = ctx.enter_context(tc.tile_pool(name="phib", bufs=6))
        phi8_pool = ctx.enter_context(tc.tile_pool(name="phi8", bufs=3))
        wb_pool = ctx.enter_context(tc.tile_pool(name="wbp", bufs=5))
        w8_pool = ctx.enter_context(tc.tile_pool(name="w8p", bufs=3))
        out_pool = ctx.enter_context(tc.tile_pool(name="outs", bufs=4))
        psum_pool = ctx.enter_context(
            tc.tile_pool(name="psum", bufs=N_OT, space="PSUM"))

        # Warm-up spin first: starts the PE busy-streak (p-state ramp)
        # while the first input DMA + phi production are still in flight.
        # DGE warm-up: a tiny transfer on each input queue absorbs the
        # cold DMA-engine ramp before the first real (critical-path) loads.
        dge_warm = const_pool.tile([2, B_SUPER], F32)
        nc.scalar.dma_start(dge_warm[:], xt[0, 0, 0:2])
        nc.sync.dma_start(dge_warm[:], xt[0, 1, 0:2])

        warm = const_pool.tile([P, 256], BF16)
        nc.gpsimd.memset(warm[:], 0.0)
        warm_ps = psum_pool.tile([P, B_SUPER], F32, tag="psum")
        for _ in range(46):
            nc.tensor.matmul(warm_ps[:, :256], lhsT=warm[:, :P],
                             rhs=warm[:], start=True, stop=True)

        # ACT bias constants: 2*g_{j+1} = j - 3 for j=0..7
        consts = const_pool.tile([P, 8], F32)
        for j in range(8):
            nc.any.memset(consts[:, j:j + 1], float(j - 3))

        # bias[o] -> [128, 8] with o = ot*128 + p; loaded lazily (first
        # needed at the first evacuation, ~110us in) so its scatter-gather
        # DMA never sits in front of the startup-critical loads.
        bias_sb = const_pool.tile([P, N_OT], F32)

        state = {"psums": None, "bias_loaded": False}

        def produce(bs, pair):
            """Weight DMAs + phi production for (bs, pair). Returns tiles."""
            # x loads first: they gate phi production (critical at startup)
            x_sbs = []
            for iik in range(2):
                ichk = 2 * pair + iik
                x_sb = x_pool.tile([P, B_SUPER], F32, tag="xin",
                                   name=f"x_{bs}_{ichk}")
                nc.scalar.dma_start(x_sb[:], xt[bs, ichk])
                x_sbs.append(x_sb)
            wbs = []
            for iik in range(2):
                ichk = 2 * pair + iik
                wb_sb = wb_pool.tile([P, NB, O], BF16, tag="wb",
                                     name=f"wb_{bs}_{ichk}")
                if bs == 0 and ichk < 2:
                    # split: first matmuls start after the ~260KB silu slice
                    nc.sync.dma_start(wb_sb[:, 6:7, :], wb[ichk, :, 6:7, :])
                    nc.sync.dma_start(wb_sb[:, 0:6, :], wb[ichk, :, 0:6, :])
                else:
                    nc.sync.dma_start(wb_sb[:], wb[ichk])
                wbs.append(wb_sb)
            w8_sb = w8_pool.tile([P, 2, 2, O], FP8, tag="w8",
                                 name=f"w8_{bs}_{pair}")
            nc.sync.dma_start(w8_sb[:], w8[pair])

            phi8_sb = phi8_pool.tile([P, 2, 2, B_SUPER], FP8, tag="p8",
                                     name=f"p8_{bs}_{pair}")
            phibs = []
            for iik in range(2):
                ichk = 2 * pair + iik
                x_sb = x_sbs[iik]
                psi = psi_pool.tile([P, 8, B_SUPER], BF16, tag="psi",
                                    name=f"psi_{bs}_{ichk}")
                phib = phib_pool.tile([P, NB, B_SUPER], BF16, tag="pb",
                                      name=f"pb_{bs}_{ichk}")
                # silu first: one ACT op after x lands -> the first super's
                # matmuls can start on the silu plane ~1.5us earlier.
                nc.scalar.activation(phib[:, 6], x_sb[:], AF.Silu)
                for j in range(8):
                    # r_j = relu(2*(g_{j+1} - x)) ; 2*g_{j+1} = j - 3
                    r = r_pool.tile([P, B_SUPER], BF16, tag="r",
                                    name=f"r_{bs}_{ichk}_{j}")
                    nc.scalar.activation(r[:], x_sb[:], AF.Relu,
                                         bias=consts[:, j:j + 1], scale=-2.0)
                    nc.vector.tensor_scalar_min(psi[:, j], r[:], 1.0)
                    if j == 7:
                        # hat_8 = relu(1 - r_7) -> fp8 plane
                        nc.scalar.activation(phi8_sb[:, iik, 1, :], r[:],
                                             AF.Relu, bias=1.0, scale=-1.0)
                # hat_0 = psi_0 -> fp8 plane (cast)
                nc.scalar.activation(phi8_sb[:, iik, 0, :], psi[:, 0],
                                     AF.Copy)
                # interior bf16 hats: hat_k = psi_k - psi_{k-1}
                for c, k in enumerate(BF_HATS):
                    nc.vector.tensor_sub(phib[:, c], psi[:, k], psi[:, k - 1])
                phibs.append(phib)
            return wbs, w8_sb, phibs, phi8_sb

        def matmuls(bs, pair, tiles):
            wbs, w8_sb, phibs, phi8_sb = tiles
            if pair == 0:
                state["psums"] = [
                    psum_pool.tile([P, B_SUPER], F32, tag="psum",
                                   name=f"ps_{bs}_{ot}")
                    for ot in range(N_OT)]
            psums = state["psums"]
            if pair < N_PAIR - 1:
                # plane-outer: earliest start once each phi plane is ready.
                # The very first block leads with silu (ready first).
                for iik in range(2):
                    order = ([6, 0, 1, 2, 3, 4, 5]
                             if bs == 0 and pair == 0 else range(NB))
                    for ci, c in enumerate(order):
                        for ot in range(N_OT):
                            nc.tensor.matmul(
                                psums[ot][:],
                                lhsT=wbs[iik][:, c, bass.ts(ot, P)],
                                rhs=phibs[iik][:, c, :],
                                start=(pair == 0 and iik == 0 and ci == 0),
                                stop=False,
                            )
                for ch in range(2):
                    for ot in range(N_OT):
                        nc.tensor.matmul(
                            psums[ot][:],
                            lhsT=w8_sb[:, :, ch, bass.ts(ot, P)],
                            rhs=phi8_sb[:, :, ch, :],
                            start=False, stop=False, perf_mode=DR,
                        )
            else:
                # last pair: ot-outer so PSUM banks finish staggered;
                # evacuation + output DMA chase each stop.
                if not state["bias_loaded"]:
                    nc.sync.dma_start(bias_sb[:],
                                      bias.rearrange("(ot p) -> p ot", p=P))
                    state["bias_loaded"] = True
                for ot in range(N_OT):
                    for iik in range(2):
                        for c in range(NB):
                            nc.tensor.matmul(
                                psums[ot][:],
                                lhsT=wbs[iik][:, c, bass.ts(ot, P)],
                                rhs=phibs[iik][:, c, :],
                                start=False, stop=False,
                            )
                    for ch in range(2):
                        nc.tensor.matmul(
                            psums[ot][:],
                            lhsT=w8_sb[:, :, ch, bass.ts(ot, P)],
                            rhs=phi8_sb[:, :, ch, :],
                            start=False, stop=(ch == 1), perf_mode=DR,
                        )
                    o_sb = out_pool.tile([P, B_SUPER], F32, tag="outs",
                                         name=f"o_{bs}_{ot}")
                    nc.vector.tensor_scalar_add(o_sb[:], psums[ot][:],
                                                bias_sb[:, ot:ot + 1])
                    eng = nc.scalar if ot % 2 == 0 else nc.sync
                    eng.dma_start(outT[ot, bs], o_sb[:])

        # ---- software pipeline: produce step i+1 before matmuls of i ----
        sched = [(bs, pair) for bs in range(N_SUPERS)
                 for pair in range(N_PAIR)]
        prev = None
        for step in sched:
            tiles = produce(*step)
            if prev is not None:
                matmuls(prev[0][0], prev[0][1], prev[1])
            prev = (step, tiles)
        matmuls(prev[0][0], prev[0][1], prev[1])

    nc.compile()
    return nc


def _get_nc():
    if "nc" not in _CACHE:
        _CACHE["nc"] = _build()
    return _CACHE["nc"]


def _stage_inputs(x, spline_weight, base_weight, bias):
    """Host-side input staging shared by kernel() and test harnesses."""
    # x[b, i] -> [core, bs, ichk, p, b_super]
    xt = np.ascontiguousarray(
        x.reshape(N_CORES, N_SUPERS, B_SUPER, N_ICHK, P)
        .transpose(0, 1, 3, 4, 2))
    sw4 = spline_weight[..., 4]                              # [O, I]
    # bf16 planes: hat{1,2,3,5,6,7} with sw_k - sw_4, then silu/base weight
    planes = [spline_weight[..., k] - sw4 for k in BF_HATS] + [base_weight]
    wb_full = np.stack(planes, axis=2)                       # [O, I, 7]
    wb_dev = np.ascontiguousarray(
        wb_full.transpose(1, 2, 0)                           # [I, 7, O]
        .reshape(N_ICHK, P, NB, O).astype(ml_dtypes.bfloat16))
    # fp8 planes: hat0/hat8 with sw_k - sw_4 -> [pair, p, iik, ch, o]
    w8_full = np.stack([(spline_weight[..., 0] - sw4).T,
                        (spline_weight[..., 8] - sw4).T], axis=1)  # [I, 2, O]
    w8_dev = np.ascontiguousarray(
        w8_full.reshape(N_PAIR, 2, P, 2, O)
        .transpose(0, 2, 1, 3, 4)                            # [pair, p, iik, ch, o]
        .astype(ml_dtypes.float8_e4m3))
    # bias fold: bias + sum_i sw[o, i, 4]
    bias_dev = (bias + sw4.sum(axis=1)).astype(np.float32)
    return xt, wb_dev, w8_dev, bias_dev


def kernel(x, spline_weight, base_weight, bias):
    x = np.asarray(x, dtype=np.float32)
    spline_weight = np.asarray(spline_weight, dtype=np.float32)
    base_weight = np.asarray(base_weight, dtype=np.float32)
    bias = np.asarray(bias, dtype=np.float32)

    nc = _get_nc()
    xt, wb_dev, w8_dev, bias_dev = _stage_inputs(
        x, spline_weight, base_weight, bias)

    in_maps = [{"xt": np.ascontiguousarray(xt[c]), "wb": wb_dev,
                "w8": w8_dev, "bias": bias_dev} for c in range(N_CORES)]
    res = run_bass_kernel_spmd(nc, in_maps, core_ids=list(range(N_CORES)))

    # outT[ot, bs, p, b] per core -> out[b, o]
    outs = []
    for c in range(N_CORES):
        oc = np.asarray(res.results[c]["outT"])
        outs.append(oc.transpose(1, 3, 0, 2).reshape(B_CORE, O))
    return np.ascontiguousarray(np.concatenate(outs, axis=0),
                                dtype=np.float32)
